# revision 9
# baseline (speedup 1.0000x reference)
"""CpxRBM translation-invariant log-psi kernel for 8 Trainium2 NeuronCores.

Computes sum(log(cosh(sym @ W.T))) where sym is the (4095, 4096) matrix of
circular shifts of v = 2*vis_states - 1 and W is (1024, 4096) complex64.

Strategy (shift-sharded, 512 shifts/core; core 7's extra shift row is masked
to zero, which contributes exactly 0 to both accumulated sums):
  - symT chunks are built ON DEVICE from a 4608-element window of the doubled
    v vector via overlapping-stride DMAs (symT[i,s] = vwin[i+s]), one DMA per
    128-row k-chunk so matmuls start almost immediately.
  - Complex matmul: sym is real, so pre = [sym @ Wr.T | sym @ Wi.T].  Host
    interleaves Wr/Wi into one (4096, 2, 1024) bf16 tensor; each (k-chunk,
    o-quarter) is one 128KB DMA and one N=512 matmul per s-tile (the moving
    operand carries both real and imag columns), fp32 PSUM accumulation.
  - log(cosh(x+iy)) elementwise: a = 2cosh(x)cos(y), b = 2sinh(x)sin(y),
      Re = 0.5*ln(a^2+b^2) - ln2
      Im = 2*atan(b / (sqrt(a^2+b^2) + a))        (exact principal atan2)
    sqrt and 1/x both via Exp/Ln so only two ACT table sets are used
    (natural_log_exp_and_others, trig_and_small); an activation-table filter
    plus explicit ordering deps keep it to 2 table loads per o-quarter.
  - Per-core output: (128, 8) fp32 partial sums; host reduces.
"""
import math
import numpy as np
import ml_dtypes
from contextlib import ExitStack

import concourse.bass as bass
import concourse.mybir as mybir
import concourse.tile as tile
from concourse import bacc
from concourse.bass_utils import run_bass_kernel_spmd
from concourse.hw_specs import get_activation_tables
import bass_rust as _bass_rust

F32 = mybir.dt.float32
BF16 = mybir.dt.bfloat16
AF = mybir.ActivationFunctionType
ALU = mybir.AluOpType

PI = float(np.pi)
VIS_N = 4096
INP_N = 4096
OUP_N = 1024
N_CORES = 8
S_PER_CORE = 512
N_KCHUNK = 32
N_QUARTER = 4
OQ = OUP_N // N_QUARTER   # 256
WIN = S_PER_CORE + INP_N  # 4608

# Only these ACT table sets may be chosen: exp+ln live together, sin+arctan
# live together -> no table thrash between Ln and Exp or Sin and Arctan.
_ALLOWED_SETS = {"natural_log_exp_and_others", "trig_and_small"}


class _Bacc(bacc.Bacc):
    def insert_act_table_loads(self):
        has_activation = any(
            isinstance(i, mybir.InstActivation)
            for b in self.main_func.blocks
            for i in b.instructions
        )
        if not has_activation:
            return
        tables = [
            (name, funcs if name in _ALLOWED_SETS else set())
            for name, funcs in get_activation_tables(self.m.arch).items()
        ]
        _bass_rust.insert_act_table_loads(self, tables)


_nc_cache = None
last_results = None


def _build_nc():
    nc = _Bacc("TRN2", target_bir_lowering=False, debug=False)

    vwin = nc.dram_tensor("vwin", [WIN], BF16, kind="ExternalInput")
    wc = nc.dram_tensor("wc", [INP_N, 2, OUP_N], BF16, kind="ExternalInput")
    msk = nc.dram_tensor("msk", [128, 1], F32, kind="ExternalInput")
    acc = nc.dram_tensor("acc", [128, 2 * N_QUARTER], F32, kind="ExternalOutput")

    with tile.TileContext(nc) as tc, ExitStack() as ctx:
        singles = ctx.enter_context(tc.tile_pool(name="singles", bufs=1))
        sympool = ctx.enter_context(tc.tile_pool(name="sympool", bufs=1))
        wpool = ctx.enter_context(tc.tile_pool(name="wpool", bufs=6))
        ppool = ctx.enter_context(tc.tile_pool(name="ppool", bufs=2, space="PSUM"))
        stage = ctx.enter_context(tc.tile_pool(name="stage", bufs=2))
        dpool = ctx.enter_context(tc.tile_pool(name="dpool", bufs=1, space="DRAM"))

        half_pi = singles.tile([128, 1], F32)
        nc.vector.memset(half_pi, PI / 2.0)
        msk_sb = singles.tile([128, 1], F32)
        nc.sync.dma_start(out=msk_sb, in_=msk[:, :])
        acc_sb = singles.tile([128, 2 * N_QUARTER], F32)

        # symT_c[p, s] = vwin[c*128 + p + s]; one tile per k-chunk so the
        # dependency tracking is exact and matmuls start as chunks land.
        # All window DMAs go on the gpsimd queue; weights use the sync and
        # scalar HWDGE queues in parallel.  {0,1} -> {-1,+1} per chunk on the
        # (idle-at-start) vector engine.
        symT = []
        for c in range(N_KCHUNK):
            st_c = sympool.tile([128, S_PER_CORE], BF16, tag=f"sym{c}", name=f"sym{c}")
            nc.gpsimd.dma_start(
                out=st_c, in_=bass.AP(vwin, c * 128, [[1, 128], [1, S_PER_CORE]])
            )
            nc.vector.tensor_scalar(st_c, st_c, 2.0, 1.0, ALU.mult, ALU.subtract)
            symT.append(st_c)

        prev_at = None
        for q in range(N_QUARTER):
            ps = ppool.tile([128, 4, 2, OQ], F32, tag="ps")
            for c in range(N_KCHUNK):
                w_t = wpool.tile([128, 2, OQ], BF16, tag="w")
                eng = nc.sync if c % 2 == 0 else nc.scalar
                eng.dma_start(
                    out=w_t, in_=wc[c * 128 : (c + 1) * 128, :, q * OQ : (q + 1) * OQ]
                )
                for st in range(4):
                    nc.tensor.matmul(
                        ps[:, st, :, :],
                        symT[c][:, st * 128 : (st + 1) * 128],
                        w_t[:, :, :],
                        start=(c == 0), stop=(c == N_KCHUNK - 1),
                    )

            # core 7's extra shift lives in partition 127 of s-tile 3; its
            # per-partition mask zeroes that pre row (a zero row contributes
            # exactly 0 to both sums).
            nc.vector.tensor_scalar(
                ps[:, 3, :, :], ps[:, 3, :, :], msk_sb, None, ALU.mult
            )

            # ---- elementwise log(cosh) + accumulation (reads ps directly) --
            g = stage.tile([128, 4, OQ], F32, tag="g")
            l = stage.tile([128, 4, OQ], F32, tag="l")
            u = stage.tile([128, 4, OQ], F32, tag="u")
            sy = stage.tile([128, 4, OQ], F32, tag="sy")
            cy = stage.tile([128, 4, OQ], F32, tag="cy")
            ep = stage.tile([128, 4, OQ], F32, tag="ep")
            em = stage.tile([128, 4, OQ], F32, tag="em")

            pxr = ps[:, :, 0, :]
            pxi = ps[:, :, 1, :]
            # range-reduce y into [-pi, pi]
            nc.vector.tensor_scalar(g, pxi, PI, 2.0 * PI, ALU.is_gt, ALU.mult)
            nc.vector.tensor_scalar(l, pxi, -PI, 2.0 * PI, ALU.is_lt, ALU.mult)
            nc.vector.scalar_tensor_tensor(u, g, -1.0, pxi, ALU.mult, ALU.add)
            nc.vector.tensor_tensor(u, u, l, ALU.add)            # u
            nc.vector.tensor_scalar(g, u, PI / 2.0, 2.0 * PI, ALU.is_gt, ALU.mult)
            nc.vector.scalar_tensor_tensor(l, g, -1.0, u, ALU.mult, ALU.add)  # ca

            i_sy = nc.scalar.activation(sy, u, AF.Sin)                # sin(y)
            i_cy = nc.scalar.activation(cy, l, AF.Sin, bias=half_pi)  # cos(y)
            i_ep = nc.scalar.activation(ep, pxr, AF.Exp)              # e^x
            i_em = nc.scalar.activation(em, pxr, AF.Exp, scale=-1.0)  # e^-x
            # keep ACT table-set order: [trig] sin,cos -> [exp/ln] block -> atan
            for a in (i_ep, i_em):
                for b in (i_sy, i_cy):
                    tile.add_dep_helper(a.ins, b.ins, reason="act-set order")
            if prev_at is not None:
                tile.add_dep_helper(i_sy.ins, prev_at.ins, reason="act-set order")
                tile.add_dep_helper(i_cy.ins, prev_at.ins, reason="act-set order")

            nc.vector.tensor_tensor(u, ep, em, ALU.add)           # t1 = 2cosh x
            nc.vector.tensor_tensor(g, ep, em, ALU.subtract)      # t2 = 2sinh x
            nc.vector.tensor_tensor(l, u, cy, ALU.mult)           # a
            nc.vector.tensor_tensor(u, g, sy, ALU.mult)           # b
            nc.vector.tensor_tensor(g, l, l, ALU.mult)            # a^2
            nc.vector.tensor_tensor(cy, u, u, ALU.mult)           # b^2
            nc.vector.tensor_tensor(sy, g, cy, ALU.add)           # q

            nc.scalar.activation(g, sy, AF.Ln, accum_out=acc_sb[:, 2 * q : 2 * q + 1])
            nc.scalar.activation(ep, g, AF.Exp, scale=0.5)        # r = sqrt(q)
            nc.vector.tensor_tensor(em, ep, l, ALU.add)           # den = r + a
            # near the branch cut fp32 rounding can push den <= 0; clamp so Ln
            # stays finite (t then blows up -> atan -> +-pi/2, correct limit).
            nc.vector.tensor_scalar(em, em, 1e-20, None, ALU.max)
            nc.scalar.activation(cy, em, AF.Ln)
            nc.scalar.activation(l, cy, AF.Exp, scale=-1.0)       # 1/den
            nc.vector.tensor_tensor(sy, u, l, ALU.mult)           # t = b/den
            prev_at = nc.scalar.activation(
                cy, sy, AF.Arctan, accum_out=acc_sb[:, 2 * q + 1 : 2 * q + 2]
            )

        nc.sync.dma_start(out=acc[:, :], in_=acc_sb)

    nc.finalize()
    return nc


def _get_nc():
    global _nc_cache
    if _nc_cache is None:
        _nc_cache = _build_nc()
    return _nc_cache


def kernel(vis_states: np.ndarray, weights: np.ndarray) -> np.ndarray:
    global last_results
    vis = np.asarray(vis_states).astype(np.float32)
    vv = np.concatenate([vis, vis]).astype(ml_dtypes.bfloat16)  # {0,1}, exact
    w = np.asarray(weights)
    wc = np.empty((INP_N, 2, OUP_N), dtype=ml_dtypes.bfloat16)
    wc[:, 0, :] = w.real.astype(np.float32).T
    wc[:, 1, :] = w.imag.astype(np.float32).T

    in_maps = []
    for c in range(N_CORES):
        s0 = c * S_PER_CORE
        m = np.ones((128, 1), np.float32)
        if c == N_CORES - 1:
            m[127, 0] = 0.0  # zero the pre row of the nonexistent 4096th shift
        in_maps.append(
            {"vwin": np.ascontiguousarray(vv[s0 : s0 + WIN]), "wc": wc, "msk": m}
        )

    nc = _get_nc()
    res = run_bass_kernel_spmd(nc, in_maps, core_ids=list(range(N_CORES)))
    last_results = res

    tot_ln = 0.0
    tot_at = 0.0
    for r in res.results:
        a = r["acc"].astype(np.float64)
        tot_ln += a[:, 0::2].sum()
        tot_at += a[:, 1::2].sum()

    n_counted = N_CORES * S_PER_CORE * OUP_N  # includes the masked zero row
    real = 0.5 * tot_ln - math.log(2.0) * n_counted
    imag = 2.0 * tot_at
    return np.array(real + 1j * imag, dtype=np.complex64)


# revision 12
# speedup vs baseline: 1.0137x; 1.0137x over previous
"""CpxRBM translation-invariant log-psi kernel for 8 Trainium2 NeuronCores.

Computes sum(log(cosh(sym @ W.T))) where sym is the (4095, 4096) matrix of
circular shifts of v = 2*vis_states - 1 and W is (1024, 4096) complex64.

Strategy (shift-sharded, 512 shifts/core; core 7's extra shift row is masked
to zero, which contributes exactly 0 to both accumulated sums):
  - symT chunks are built ON DEVICE from a 4608-element window of the doubled
    v vector via overlapping-stride DMAs (symT[i,s] = vwin[i+s]), one DMA per
    128-row k-chunk so matmuls start almost immediately.
  - Complex matmul: sym is real, so pre = [sym @ Wr.T | sym @ Wi.T].  Host
    interleaves Wr/Wi into one (4096, 2, 1024) bf16 tensor; each (k-chunk,
    o-quarter) is one 128KB DMA and one N=512 matmul per s-tile (the moving
    operand carries both real and imag columns), fp32 PSUM accumulation.
  - log(cosh(x+iy)) elementwise: a = 2cosh(x)cos(y), b = 2sinh(x)sin(y),
      Re = 0.5*ln(a^2+b^2) - ln2
      Im = 2*atan(b / (sqrt(a^2+b^2) + a))        (exact principal atan2)
    sqrt and 1/x both via Exp/Ln so only two ACT table sets are used
    (natural_log_exp_and_others, trig_and_small); an activation-table filter
    plus explicit ordering deps keep it to 2 table loads per o-quarter.
  - Per-core output: (128, 8) fp32 partial sums; host reduces.
"""
import math
import numpy as np
import ml_dtypes
from contextlib import ExitStack

import concourse.bass as bass
import concourse.mybir as mybir
import concourse.tile as tile
from concourse import bacc
from concourse.bass_utils import run_bass_kernel_spmd
from concourse.hw_specs import get_activation_tables
import bass_rust as _bass_rust

F32 = mybir.dt.float32
BF16 = mybir.dt.bfloat16
AF = mybir.ActivationFunctionType
ALU = mybir.AluOpType

PI = float(np.pi)
VIS_N = 4096
INP_N = 4096
OUP_N = 1024
N_CORES = 8
S_PER_CORE = 512
N_KCHUNK = 32
N_QUARTER = 4
OQ = OUP_N // N_QUARTER   # 256
WIN = S_PER_CORE + INP_N  # 4608

# Only these ACT table sets may be chosen: exp+ln live together, sin+arctan
# live together -> no table thrash between Ln and Exp or Sin and Arctan.
_ALLOWED_SETS = {"natural_log_exp_and_others", "trig_and_small"}


class _Bacc(bacc.Bacc):
    def insert_act_table_loads(self):
        has_activation = any(
            isinstance(i, mybir.InstActivation)
            for b in self.main_func.blocks
            for i in b.instructions
        )
        if not has_activation:
            return
        tables = [
            (name, funcs if name in _ALLOWED_SETS else set())
            for name, funcs in get_activation_tables(self.m.arch).items()
        ]
        _bass_rust.insert_act_table_loads(self, tables)


_nc_cache = None
last_results = None


def _build_nc():
    nc = _Bacc("TRN2", target_bir_lowering=False, debug=False)

    vwin = nc.dram_tensor("vwin", [WIN], BF16, kind="ExternalInput")
    wc = nc.dram_tensor("wc", [INP_N, 2, OUP_N], BF16, kind="ExternalInput")
    msk = nc.dram_tensor("msk", [128, 1], F32, kind="ExternalInput")
    acc = nc.dram_tensor("acc", [128, 2 * N_QUARTER], F32, kind="ExternalOutput")

    with tile.TileContext(nc) as tc, ExitStack() as ctx:
        singles = ctx.enter_context(tc.tile_pool(name="singles", bufs=1))
        sympool = ctx.enter_context(tc.tile_pool(name="sympool", bufs=1))
        wpool = ctx.enter_context(tc.tile_pool(name="wpool", bufs=6))
        ppool = ctx.enter_context(tc.tile_pool(name="ppool", bufs=2, space="PSUM"))
        stage = ctx.enter_context(tc.tile_pool(name="stage", bufs=2))
        dpool = ctx.enter_context(tc.tile_pool(name="dpool", bufs=1, space="DRAM"))

        half_pi = singles.tile([128, 1], F32)
        nc.vector.memset(half_pi, PI / 2.0)
        msk_sb = singles.tile([128, 1], F32)
        nc.sync.dma_start(out=msk_sb, in_=msk[:, :])
        acc_sb = singles.tile([128, 2 * N_QUARTER], F32)

        # symT_c[p, s] = vwin[c*128 + p + s]; one tile per k-chunk so the
        # dependency tracking is exact and matmuls start as chunks land.
        # Interleave sym-window and first-quarter weight DMAs on the two
        # HWDGE queues (sync/scalar, crossed) so chunk c's operands both
        # arrive at ~0.65us*c.  {0,1} -> {-1,+1} per chunk on the vector
        # engine, which is idle at the start.
        symT = []
        w_q0 = []
        for c in range(N_KCHUNK):
            st_c = sympool.tile([128, S_PER_CORE], BF16, tag=f"sym{c}", name=f"sym{c}")
            (nc.sync if c % 2 == 0 else nc.scalar).dma_start(
                out=st_c, in_=bass.AP(vwin, c * 128, [[1, 128], [1, S_PER_CORE]])
            )
            nc.vector.tensor_scalar(st_c, st_c, 2.0, 1.0, ALU.mult, ALU.subtract)
            symT.append(st_c)
            w_t = wpool.tile([128, 2, OQ], BF16, tag=f"wq0_{c}", name=f"wq0_{c}", bufs=1)
            (nc.scalar if c % 2 == 0 else nc.sync).dma_start(
                out=w_t, in_=wc[c * 128 : (c + 1) * 128, :, 0:OQ]
            )
            w_q0.append(w_t)

        prev_at = None
        for q in range(N_QUARTER):
            ps = ppool.tile([128, 4, 2, OQ], F32, tag="ps")
            for c in range(N_KCHUNK):
                if q == 0:
                    w_t = w_q0[c]
                else:
                    w_t = wpool.tile([128, 2, OQ], BF16, tag="w")
                    eng = nc.sync if c % 2 == 0 else nc.scalar
                    eng.dma_start(
                        out=w_t,
                        in_=wc[c * 128 : (c + 1) * 128, :, q * OQ : (q + 1) * OQ],
                    )
                for st in range(4):
                    nc.tensor.matmul(
                        ps[:, st, :, :],
                        symT[c][:, st * 128 : (st + 1) * 128],
                        w_t[:, :, :],
                        start=(c == 0), stop=(c == N_KCHUNK - 1),
                    )

            # core 7's extra shift lives in partition 127 of s-tile 3; its
            # per-partition mask zeroes that pre row (a zero row contributes
            # exactly 0 to both sums).
            nc.vector.tensor_scalar(
                ps[:, 3, :, :], ps[:, 3, :, :], msk_sb, None, ALU.mult
            )

            # ---- elementwise log(cosh) + accumulation ----
            xr = stage.tile([128, 4, OQ], F32, tag="xr")
            xi = stage.tile([128, 4, OQ], F32, tag="xi")
            g = stage.tile([128, 4, OQ], F32, tag="g")
            l = stage.tile([128, 4, OQ], F32, tag="l")
            u = stage.tile([128, 4, OQ], F32, tag="u")
            sy = stage.tile([128, 4, OQ], F32, tag="sy")
            cy = stage.tile([128, 4, OQ], F32, tag="cy")
            ep = stage.tile([128, 4, OQ], F32, tag="ep")
            em = stage.tile([128, 4, OQ], F32, tag="em")

            # copy out promptly so the psum banks free up for the next quarter
            nc.vector.tensor_copy(xr, ps[:, :, 0, :])
            nc.vector.tensor_copy(xi, ps[:, :, 1, :])

            # range-reduce y into [-pi, pi]
            nc.vector.tensor_scalar(g, xi, PI, 2.0 * PI, ALU.is_gt, ALU.mult)
            nc.vector.tensor_scalar(l, xi, -PI, 2.0 * PI, ALU.is_lt, ALU.mult)
            nc.vector.scalar_tensor_tensor(u, g, -1.0, xi, ALU.mult, ALU.add)
            nc.vector.tensor_tensor(u, u, l, ALU.add)            # u
            nc.vector.tensor_scalar(g, u, PI / 2.0, 2.0 * PI, ALU.is_gt, ALU.mult)
            nc.vector.scalar_tensor_tensor(l, g, -1.0, u, ALU.mult, ALU.add)  # ca

            i_sy = nc.scalar.activation(sy, u, AF.Sin)                # sin(y)
            i_cy = nc.scalar.activation(cy, l, AF.Sin, bias=half_pi)  # cos(y)
            i_ep = nc.scalar.activation(ep, xr, AF.Exp)               # e^x
            i_em = nc.scalar.activation(em, xr, AF.Exp, scale=-1.0)   # e^-x
            # keep ACT table-set order: [trig] sin,cos -> [exp/ln] block -> atan
            for a in (i_ep, i_em):
                for b in (i_sy, i_cy):
                    tile.add_dep_helper(a.ins, b.ins, reason="act-set order")
            if prev_at is not None:
                tile.add_dep_helper(i_sy.ins, prev_at.ins, reason="act-set order")
                tile.add_dep_helper(i_cy.ins, prev_at.ins, reason="act-set order")

            nc.vector.tensor_tensor(u, ep, em, ALU.add)           # t1 = 2cosh x
            nc.vector.tensor_tensor(g, ep, em, ALU.subtract)      # t2 = 2sinh x
            nc.vector.tensor_tensor(l, u, cy, ALU.mult)           # a
            nc.vector.tensor_tensor(u, g, sy, ALU.mult)           # b
            nc.vector.tensor_tensor(g, l, l, ALU.mult)            # a^2
            nc.vector.tensor_tensor(cy, u, u, ALU.mult)           # b^2
            nc.vector.tensor_tensor(sy, g, cy, ALU.add)           # q

            nc.scalar.activation(g, sy, AF.Ln, accum_out=acc_sb[:, 2 * q : 2 * q + 1])
            nc.scalar.activation(ep, g, AF.Exp, scale=0.5)        # r = sqrt(q)
            nc.vector.tensor_tensor(em, ep, l, ALU.add)           # den = r + a
            # near the branch cut fp32 rounding can push den <= 0; clamp so Ln
            # stays finite (t then blows up -> atan -> +-pi/2, correct limit).
            nc.vector.tensor_scalar(em, em, 1e-20, None, ALU.max)
            nc.scalar.activation(cy, em, AF.Ln)
            nc.scalar.activation(l, cy, AF.Exp, scale=-1.0)       # 1/den
            nc.vector.tensor_tensor(sy, u, l, ALU.mult)           # t = b/den
            prev_at = nc.scalar.activation(
                cy, sy, AF.Arctan, accum_out=acc_sb[:, 2 * q + 1 : 2 * q + 2]
            )

        nc.sync.dma_start(out=acc[:, :], in_=acc_sb)

    nc.finalize()
    return nc


def _get_nc():
    global _nc_cache
    if _nc_cache is None:
        _nc_cache = _build_nc()
    return _nc_cache


def kernel(vis_states: np.ndarray, weights: np.ndarray) -> np.ndarray:
    global last_results
    vis = np.asarray(vis_states).astype(np.float32)
    vv = np.concatenate([vis, vis]).astype(ml_dtypes.bfloat16)  # {0,1}, exact
    w = np.asarray(weights)
    wc = np.empty((INP_N, 2, OUP_N), dtype=ml_dtypes.bfloat16)
    wc[:, 0, :] = w.real.astype(np.float32).T
    wc[:, 1, :] = w.imag.astype(np.float32).T

    in_maps = []
    for c in range(N_CORES):
        s0 = c * S_PER_CORE
        m = np.ones((128, 1), np.float32)
        if c == N_CORES - 1:
            m[127, 0] = 0.0  # zero the pre row of the nonexistent 4096th shift
        in_maps.append(
            {"vwin": np.ascontiguousarray(vv[s0 : s0 + WIN]), "wc": wc, "msk": m}
        )

    nc = _get_nc()
    res = run_bass_kernel_spmd(nc, in_maps, core_ids=list(range(N_CORES)))
    last_results = res

    tot_ln = 0.0
    tot_at = 0.0
    for r in res.results:
        a = r["acc"].astype(np.float64)
        tot_ln += a[:, 0::2].sum()
        tot_at += a[:, 1::2].sum()

    n_counted = N_CORES * S_PER_CORE * OUP_N  # includes the masked zero row
    real = 0.5 * tot_ln - math.log(2.0) * n_counted
    imag = 2.0 * tot_at
    return np.array(real + 1j * imag, dtype=np.complex64)
